# revision 8
# baseline (speedup 1.0000x reference)
"""Trainium2 Bass kernel: GNN message passing (RBF all-pairs + embed einsum + linear).

Strategy (8 NeuronCores, SPMD):
  The reference output is atom_features.sum(axis=0) -- linear in everything
  after the RBF tensor -- so the whole network collapses to:
      s[j,k]  = sum_i exp(-gamma_k * ||pc_i - pc_j||^2)        (incl. i==j)
      ns[k,:] = sum_j (s[j,k] - 1) * E[j,:]                     (diag removed)
      out     = concat(sumE, ns.flatten()) @ W + N*b
  Each core owns a 512-row j-slice, computes s over all i via one augmented
  matmul (distance) + exp on ScalarE / chained multiplies on VectorE (fused
  row-reduce), histograms s against the one-hot of atomic numbers on the
  TensorEngine, and applies the embedding + final linear locally.  Per-core
  [512] partial outputs sum to the exact full result (host-side unshard).
"""

import os
import sys

import numpy as np

N = 4096
NCORES = 8
JC = N // NCORES          # j rows per core
P = 128
NB = JC // P              # j-blocks of 128 partitions per core
KRBF = 16
EMB = 64
VOCAB = 128
MD = 512
FIN = EMB + EMB * KRBF    # 1088
CHUNK = 2048              # i-chunk (PSUM: [128, 2048] f32 = 4 banks)
NCHUNK = N // CHUNK


def _import_concourse():
    try:
        import concourse  # noqa: F401
        return
    except ImportError:
        pass
    for p in ("/opt/trn_rl_repo", "/root/.axon_site/_ro/trn_rl_repo"):
        if os.path.isdir(os.path.join(p, "concourse")):
            sys.path.insert(0, p)
            import concourse  # noqa: F401
            return
    raise ImportError("cannot locate the concourse (bass) package")


def _plan_k(gamma: np.ndarray):
    """Anchor/chain split of the 16 RBF exponents.

    If gamma is uniformly spaced (it is: linspace), odd k's are computed on
    the VectorEngine as rbf[k] = rbf[k-1] * exp(-dg*D); otherwise fall back
    to 16 direct exps on ScalarE."""
    g = np.asarray(gamma, np.float64).reshape(-1)
    assert g.shape[0] == KRBF
    dif = np.diff(g)
    if np.allclose(dif, dif[0], rtol=1e-5, atol=1e-7):
        anchors = list(range(0, KRBF, 2))
        chains = list(range(1, KRBF, 2))
        return anchors, chains, float(dif[0])
    return list(range(KRBF)), [], 0.0


def _prepare_in_maps(atomic_numbers, positions, embed_table, W, b):
    pos = np.asarray(positions, np.float32)
    pc = pos - pos.mean(axis=0, keepdims=True)
    nrm = (pc * pc).sum(axis=1).astype(np.float32)
    ones = np.ones(N, np.float32)
    A_full = np.ascontiguousarray(
        np.stack([pc[:, 0], pc[:, 1], pc[:, 2], nrm, ones]))            # [5, N]
    B_full = np.ascontiguousarray(
        np.stack([-2 * pc[:, 0], -2 * pc[:, 1], -2 * pc[:, 2], ones, nrm]))
    a = np.asarray(atomic_numbers).astype(np.int64).reshape(N)
    T = np.ascontiguousarray(np.asarray(embed_table, np.float32))
    Wf = np.ascontiguousarray(np.asarray(W, np.float32))
    bf = np.ascontiguousarray(np.asarray(b, np.float32).reshape(1, MD))
    in_maps = []
    for c in range(NCORES):
        js = slice(c * JC, (c + 1) * JC)
        ac = np.ascontiguousarray(
            a[js].reshape(NB, P).T.astype(np.float32))                  # [128, NB]
        in_maps.append({
            "A": np.ascontiguousarray(A_full[:, js]),
            "B": B_full,
            "AC": ac,
            "T": T,
            "W": Wf,
            "BB": bf,
        })
    return in_maps


def _build(gamma, anchors, chains, dg):
    from contextlib import ExitStack

    import concourse.mybir as mybir
    import concourse.tile as tile
    from concourse import bacc
    from concourse.masks import make_identity

    f32 = mybir.dt.float32
    Alu = mybir.AluOpType
    Act = mybir.ActivationFunctionType

    g = [float(x) for x in np.asarray(gamma, np.float64).reshape(-1)]

    nc = bacc.Bacc("TRN2", target_bir_lowering=False, debug=False,
                   num_devices=NCORES)

    A_ext = nc.declare_dram_parameter("A", [5, JC], f32, isOutput=False)
    B_ext = nc.declare_dram_parameter("B", [5, N], f32, isOutput=False)
    AC_ext = nc.declare_dram_parameter("AC", [P, NB], f32, isOutput=False)
    T_ext = nc.declare_dram_parameter("T", [VOCAB, EMB], f32, isOutput=False)
    W_ext = nc.declare_dram_parameter("W", [FIN, MD], f32, isOutput=False)
    BB_ext = nc.declare_dram_parameter("BB", [1, MD], f32, isOutput=False)
    out_ext = nc.declare_dram_parameter("out", [1, MD], f32, isOutput=True)

    with tile.TileContext(nc) as tc, ExitStack() as ctx:
        consts = ctx.enter_context(tc.tile_pool(name="consts", bufs=1))
        sb = ctx.enter_context(tc.tile_pool(name="sb", bufs=2))
        # NOTE: 8 accum-activations per chunk rotating through fewer SBUF
        # slots than anchors-in-flight crashed the exec unit on hardware
        # (NRT_EXEC_UNIT_UNRECOVERABLE); bufs >= anchors-per-chunk avoids
        # intra-chunk slot reuse and runs clean.  Chain outputs are pure
        # sinks (the row-sum rides the fp32 ALU result), so they can be
        # bf16 to save SBUF.
        rbfa_pool = ctx.enter_context(tc.tile_pool(name="rbfa", bufs=8))
        rbfc_pool = ctx.enter_context(tc.tile_pool(name="rbfc", bufs=10))
        r_pool = ctx.enter_context(tc.tile_pool(name="rp", bufs=2))
        ps = ctx.enter_context(tc.tile_pool(name="ps", bufs=2, space="PSUM"))

        # ---- inputs / constants ----
        A_sb = consts.tile([5, JC], f32)
        nc.sync.dma_start(A_sb[:], A_ext[:, :])
        B_sb = consts.tile([5, N], f32)
        nc.sync.dma_start(B_sb[:], B_ext[:, :])
        AC_sb = consts.tile([P, NB], f32)
        nc.sync.dma_start(AC_sb[:], AC_ext[:, :])
        T_sb = consts.tile([VOCAB, EMB], f32)
        nc.sync.dma_start(T_sb[:], T_ext[:, :])
        W_sb = consts.tile([EMB, (KRBF + 1) * MD], f32)
        for cc in range(KRBF + 1):
            nc.sync.dma_start(W_sb[:, cc * MD:(cc + 1) * MD],
                              W_ext[cc * EMB:(cc + 1) * EMB, :])
        b_sb = consts.tile([1, MD], f32)
        nc.sync.dma_start(b_sb[:], BB_ext[:, :])

        iota_i = consts.tile([P, P], mybir.dt.int32)
        nc.gpsimd.iota(iota_i[:], pattern=[[1, P]], base=0, channel_multiplier=0)
        iota_f = consts.tile([P, P], f32)
        nc.vector.tensor_copy(iota_f[:], iota_i[:])
        ident = consts.tile([32, 32], f32)
        make_identity(nc, ident[:])
        G_sb = consts.tile([P, KRBF + 1], f32)

        # ---- main loop: per j-block of 128 partitions ----
        for jb in range(NB):
            s_all = sb.tile([P, KRBF * NCHUNK], f32, tag="s_all")
            oh = sb.tile([P, P], f32, tag="oh")
            nc.vector.tensor_scalar(oh[:], iota_f[:], AC_sb[:, jb:jb + 1],
                                    None, Alu.is_equal)
            for ci in range(NCHUNK):
                Dp = ps.tile([P, CHUNK], f32, tag="ps")
                for s0 in range(0, CHUNK, 512):
                    nc.tensor.matmul(
                        Dp[:, s0:s0 + 512],
                        A_sb[:, jb * P:(jb + 1) * P],
                        B_sb[:, ci * CHUNK + s0: ci * CHUNK + s0 + 512],
                        start=True, stop=True)
                cur = {}
                if chains:
                    r_t = r_pool.tile([P, CHUNK], f32, tag="r")
                    nc.scalar.activation(r_t[:], Dp[:], Act.Exp, scale=-dg)
                # Interleave anchor (ScalarE exp) with its dependent chain
                # (VectorE multiply) so anchor-tile lifetimes stay short.
                plan = []
                for k in anchors:
                    plan.append(("a", k))
                    for k2 in chains:
                        if k2 - 1 == k:
                            plan.append(("c", k2))
                for kind, k in plan:
                    col = k * NCHUNK + ci
                    if kind == "a":
                        t = rbfa_pool.tile([P, CHUNK], f32, tag="rbf_a")
                        nc.scalar.activation(t[:], Dp[:], Act.Exp, scale=-g[k],
                                             accum_out=s_all[:, col:col + 1])
                    else:
                        # NB: tensor_tensor_reduce + accum_out crashes the
                        # device in this pattern; scalar_tensor_tensor with
                        # accum_out computes the same product + row-sum and
                        # runs clean.
                        t = rbfc_pool.tile([P, CHUNK], mybir.dt.bfloat16,
                                           tag="rbf_c")
                        nc.vector.scalar_tensor_tensor(
                            out=t[:], in0=cur[k - 1][:], scalar=1.0,
                            in1=r_t[:], op0=Alu.mult, op1=Alu.mult,
                            accum_out=s_all[:, col:col + 1])
                    cur[k] = t
            s17 = sb.tile([P, KRBF + 1], f32, tag="s17")
            nc.vector.tensor_reduce(
                s17[:, 0:KRBF],
                s_all[:].rearrange("p (k c) -> p k c", c=NCHUNK),
                axis=mybir.AxisListType.X, op=Alu.add)
            nc.vector.memset(s17[:, KRBF:KRBF + 1], 1.0)
            hist = ps.tile([P, KRBF + 1], f32, tag="ps")
            nc.tensor.matmul(hist[:], oh[:], s17[:], start=True, stop=True)
            if jb == 0:
                nc.vector.tensor_copy(G_sb[:], hist[:])
            else:
                nc.vector.tensor_tensor(out=G_sb[:], in0=G_sb[:], in1=hist[:],
                                        op=Alu.add)

        # ---- epilogue: res = G^T @ T, transpose, subtract sumE, final linear ----
        res = ps.tile([KRBF + 1, EMB], f32, tag="ps")
        nc.tensor.matmul(res[:], G_sb[:], T_sb[:], start=True, stop=True)
        res_sb = sb.tile([KRBF + 1, EMB], f32, tag="res")
        nc.vector.tensor_copy(res_sb[:], res[:])
        resT = ps.tile([EMB, KRBF + 1], f32, tag="ps")
        nc.tensor.transpose(resT[:], res_sb[:], ident[0:KRBF + 1, 0:KRBF + 1])
        resT_sb = sb.tile([EMB, KRBF + 1], f32, tag="resT")
        nc.vector.tensor_copy(resT_sb[:], resT[:])
        fT = sb.tile([EMB, KRBF + 1], f32, tag="fT")
        nc.vector.tensor_scalar(fT[:, 0:KRBF], resT_sb[:, 0:KRBF],
                                resT_sb[:, KRBF:KRBF + 1], None, Alu.subtract)
        nc.vector.tensor_copy(fT[:, KRBF:KRBF + 1], resT_sb[:, KRBF:KRBF + 1])

        outp = ps.tile([1, MD], f32, tag="ps")
        for cc in range(KRBF + 1):
            col = KRBF if cc == 0 else cc - 1
            nc.tensor.matmul(outp[:],
                             fT[:, col:col + 1],
                             W_sb[:, cc * MD:(cc + 1) * MD],
                             start=(cc == 0), stop=(cc == KRBF))
        bsc = sb.tile([1, MD], f32, tag="bsc")
        nc.scalar.mul(bsc[:], b_sb[:], float(N) / NCORES)
        out_sb = sb.tile([1, MD], f32, tag="outsb")
        nc.vector.tensor_tensor(out=out_sb[:], in0=outp[:], in1=bsc[:],
                                op=Alu.add)
        nc.sync.dma_start(out_ext[:, :], out_sb[:])

    nc.compile()
    return nc


def _install_ntff_hook_shim():
    """Provide antenv.axon_hooks if the image's antenv lacks it.

    concourse's trace path (run_bass_kernel_spmd(trace=True) under axon)
    imports get_axon_ntff_profile_hook from there; the hook drives NRT
    profiling through libaxon_pjrt.so's C ABI (same contract the boot
    script uses)."""
    try:
        from antenv.axon_hooks import get_axon_ntff_profile_hook  # noqa: F401
        return
    except ImportError:
        pass
    import contextlib
    import ctypes
    import types

    so_path = os.environ.get("PJRT_LIBRARY_PATH", "/opt/axon/libaxon_pjrt.so")
    hook = None
    try:
        lib = ctypes.CDLL(so_path)
        if hasattr(lib, "axon_start_nrt_profile"):
            lib.axon_start_nrt_profile.argtypes = [
                ctypes.POINTER(ctypes.c_int64), ctypes.c_size_t]
            lib.axon_start_nrt_profile.restype = ctypes.c_int64
            lib.axon_stop_nrt_profile.argtypes = [ctypes.c_char_p]
            lib.axon_stop_nrt_profile.restype = ctypes.c_int64

            @contextlib.contextmanager
            def _hook(output_dir, device_ids):
                import jax
                jax.devices()
                if device_ids:
                    ids = (ctypes.c_int64 * len(device_ids))(*device_ids)
                    rc = lib.axon_start_nrt_profile(ids, len(device_ids))
                else:
                    rc = lib.axon_start_nrt_profile(None, 0)
                if rc != 0:
                    raise RuntimeError(f"axon_start_nrt_profile rc={rc}")
                try:
                    yield
                finally:
                    n = lib.axon_stop_nrt_profile(str(output_dir).encode())
                    print(f"ntff profile: {n} file(s) -> {output_dir}",
                          file=sys.stderr)

            hook = _hook
    except OSError:
        hook = None

    import antenv
    mod = types.ModuleType("antenv.axon_hooks")
    mod._hook = hook
    mod.get_axon_ntff_profile_hook = lambda: mod._hook

    def _set(h):
        mod._hook = h

    mod.set_axon_ntff_profile_hook = _set
    sys.modules["antenv.axon_hooks"] = mod
    antenv.axon_hooks = mod


def _run(inputs, trace=False):
    """Build + run on 8 NeuronCores. Returns (out[512] f32, exec_time_ns|None)."""
    _import_concourse()
    if trace:
        _install_ntff_hook_shim()
        from concourse import bass_utils as _bu
        _bu.upload_artifacts = lambda tmpdir: "local://" + str(tmpdir)
    from concourse.bass_utils import run_bass_kernel_spmd

    gamma = np.asarray(inputs["gamma"], np.float32).reshape(-1)
    anchors, chains, dg = _plan_k(gamma)
    in_maps = _prepare_in_maps(inputs["atomic_numbers"], inputs["positions"],
                               inputs["embed_table"], inputs["W"], inputs["b"])
    nc = _build(gamma, anchors, chains, dg)
    res = run_bass_kernel_spmd(nc, in_maps, core_ids=list(range(NCORES)),
                               trace=trace)
    out = np.zeros(MD, np.float32)
    for r in res.results:
        out += np.asarray(r["out"], np.float32).reshape(-1)
    return out, res.exec_time_ns


def kernel(**inputs) -> np.ndarray:
    out, _ = _run(inputs, trace=False)
    return out


# revision 13
# speedup vs baseline: 1.2654x; 1.2654x over previous
"""Trainium2 Bass kernel: GNN message passing (RBF all-pairs + embed einsum + linear).

Strategy (8 NeuronCores, SPMD):
  The reference output is atom_features.sum(axis=0) -- linear in everything
  after the RBF tensor -- so the whole network collapses to:
      s[j,k]  = sum_i exp(-gamma_k * ||pc_i - pc_j||^2)        (incl. i==j)
      ns[k,:] = sum_j (s[j,k] - 1) * E[j,:]                     (diag removed)
      out     = concat(sumE, ns.flatten()) @ W + N*b
  Each core owns a 512-row j-slice, computes s over all i via one augmented
  matmul (distance) + exp on ScalarE / chained multiplies on VectorE (fused
  row-reduce), histograms s against the one-hot of atomic numbers on the
  TensorEngine, and applies the embedding + final linear locally.  Per-core
  [512] partial outputs sum to the exact full result (host-side unshard).
"""

import os
import sys

import numpy as np

N = 4096
NCORES = 8
JC = N // NCORES          # j rows per core
P = 128
NB = JC // P              # j-blocks of 128 partitions per core
KRBF = 16
EMB = 64
VOCAB = 128
MD = 512
FIN = EMB + EMB * KRBF    # 1088
CHUNK = 2048              # i-chunk (PSUM: [128, 2048] f32 = 4 banks)
NCHUNK = N // CHUNK
ANCHOR_STRIDE = 2         # every ANCHOR_STRIDE-th k via ScalarE exp, rest chained on DVE


def _import_concourse():
    try:
        import concourse  # noqa: F401
        return
    except ImportError:
        pass
    for p in ("/opt/trn_rl_repo", "/root/.axon_site/_ro/trn_rl_repo"):
        if os.path.isdir(os.path.join(p, "concourse")):
            sys.path.insert(0, p)
            import concourse  # noqa: F401
            return
    raise ImportError("cannot locate the concourse (bass) package")


def _plan_k(gamma: np.ndarray):
    """Anchor/chain split of the 16 RBF exponents.

    If gamma is uniformly spaced (it is: linspace), odd k's are computed on
    the VectorEngine as rbf[k] = rbf[k-1] * exp(-dg*D); otherwise fall back
    to 16 direct exps on ScalarE."""
    g = np.asarray(gamma, np.float64).reshape(-1)
    assert g.shape[0] == KRBF
    dif = np.diff(g)
    if np.allclose(dif, dif[0], rtol=1e-5, atol=1e-7):
        anchors = list(range(0, KRBF, ANCHOR_STRIDE))
        chains = [k for k in range(KRBF) if k not in anchors]
        return anchors, chains, float(dif[0])
    return list(range(KRBF)), [], 0.0


def _prepare_in_maps(atomic_numbers, positions, embed_table, W, b):
    pos = np.asarray(positions, np.float32)
    pc = pos - pos.mean(axis=0, keepdims=True)
    nrm = (pc * pc).sum(axis=1).astype(np.float32)
    ones = np.ones(N, np.float32)
    A_full = np.ascontiguousarray(
        np.stack([pc[:, 0], pc[:, 1], pc[:, 2], nrm, ones]))            # [5, N]
    B_full = np.ascontiguousarray(
        np.stack([-2 * pc[:, 0], -2 * pc[:, 1], -2 * pc[:, 2], ones, nrm]))
    a = np.asarray(atomic_numbers).astype(np.int64).reshape(N)
    T = np.ascontiguousarray(np.asarray(embed_table, np.float32))
    Wf = np.asarray(W, np.float32)
    # W128: pair the 17 64-row f-chunks (ns_0..ns_15, sumE) onto 128
    # partitions so the final linear is 9 K=128 matmuls instead of 17 K=64.
    # f-chunk i: ns_i -> W rows [64+64i, 128+64i); sumE -> W rows [0, 64).
    def _wchunk(i):
        if i < KRBF:
            return Wf[EMB + EMB * i: 2 * EMB + EMB * i, :]
        if i == KRBF:
            return Wf[0:EMB, :]
        return np.zeros((EMB, MD), np.float32)
    W128 = np.empty((P, 9 * MD), np.float32)
    for c in range(9):
        W128[0:EMB, c * MD:(c + 1) * MD] = _wchunk(2 * c)
        W128[EMB:P, c * MD:(c + 1) * MD] = _wchunk(2 * c + 1)
    W128 = np.ascontiguousarray(W128)
    bf = np.ascontiguousarray(np.asarray(b, np.float32).reshape(1, MD))
    in_maps = []
    for c in range(NCORES):
        js = slice(c * JC, (c + 1) * JC)
        ac = np.ascontiguousarray(
            a[js].reshape(NB, P).T.astype(np.float32))                  # [128, NB]
        in_maps.append({
            "A": np.ascontiguousarray(A_full[:, js]),
            "B": B_full,
            "AC": ac,
            "T": T,
            "W": W128,
            "BB": bf,
        })
    return in_maps


def _build(gamma, anchors, chains, dg):
    from contextlib import ExitStack

    import concourse.mybir as mybir
    import concourse.tile as tile
    from concourse import bacc
    from concourse.masks import make_identity

    f32 = mybir.dt.float32
    bf16 = mybir.dt.bfloat16
    Alu = mybir.AluOpType
    Act = mybir.ActivationFunctionType

    g = [float(x) for x in np.asarray(gamma, np.float64).reshape(-1)]

    nc = bacc.Bacc("TRN2", target_bir_lowering=False, debug=False,
                   num_devices=NCORES)

    A_ext = nc.declare_dram_parameter("A", [5, JC], f32, isOutput=False)
    B_ext = nc.declare_dram_parameter("B", [5, N], f32, isOutput=False)
    AC_ext = nc.declare_dram_parameter("AC", [P, NB], f32, isOutput=False)
    T_ext = nc.declare_dram_parameter("T", [VOCAB, EMB], f32, isOutput=False)
    W_ext = nc.declare_dram_parameter("W", [P, 9 * MD], f32, isOutput=False)
    BB_ext = nc.declare_dram_parameter("BB", [1, MD], f32, isOutput=False)
    out_ext = nc.declare_dram_parameter("out", [1, MD], f32, isOutput=True)

    with tile.TileContext(nc) as tc, ExitStack() as ctx:
        consts = ctx.enter_context(tc.tile_pool(name="consts", bufs=1))
        sb = ctx.enter_context(tc.tile_pool(name="sb", bufs=2))
        # NOTE: 8 accum-activations per chunk rotating through fewer SBUF
        # slots than anchors-in-flight crashed the exec unit on hardware
        # (NRT_EXEC_UNIT_UNRECOVERABLE); bufs >= anchors-per-chunk avoids
        # intra-chunk slot reuse and runs clean.  Chain outputs are pure
        # sinks (the row-sum rides the fp32 ALU result), so they can be
        # bf16 to save SBUF.
        rbfa_pool = ctx.enter_context(tc.tile_pool(name="rbfa", bufs=8))
        rbfc_pool = ctx.enter_context(tc.tile_pool(name="rbfc", bufs=10))
        r_pool = ctx.enter_context(tc.tile_pool(name="rp", bufs=2))
        ps = ctx.enter_context(tc.tile_pool(name="ps", bufs=2, space="PSUM"))

        # ---- inputs / constants ----
        A_sb = consts.tile([5, JC], f32)
        nc.sync.dma_start(A_sb[:], A_ext[:, :])
        B_sb = consts.tile([5, N], f32)
        nc.sync.dma_start(B_sb[:], B_ext[:, :])
        AC_sb = consts.tile([P, NB], f32)
        nc.sync.dma_start(AC_sb[:], AC_ext[:, :])
        T_sb = consts.tile([VOCAB, EMB], f32)
        nc.sync.dma_start(T_sb[:], T_ext[:, :])
        W_sb = consts.tile([P, 9 * MD], f32)
        nc.sync.dma_start(W_sb[:], W_ext[:, :])
        b_sb = consts.tile([1, MD], f32)
        nc.sync.dma_start(b_sb[:], BB_ext[:, :])

        iota_i = consts.tile([P, P], mybir.dt.int32)
        nc.gpsimd.iota(iota_i[:], pattern=[[1, P]], base=0, channel_multiplier=0)
        iota_f = consts.tile([P, P], f32)
        nc.vector.tensor_copy(iota_f[:], iota_i[:])
        ident = consts.tile([32, 32], f32)
        make_identity(nc, ident[:])
        G_sb = consts.tile([P, KRBF + 1], f32)
        # Per-k schedule: each anchor is followed by the chain run it feeds
        # (chain k multiplies k-1's tile by r on the VectorEngine).
        plan = []
        for k in range(KRBF):
            plan.append(("a" if k in anchors else "c", k))

        # ---- main loop over (j-block, i-chunk) jobs ----
        # D for job t+1 is emitted before job t's exp/chain work so the
        # TensorEngine fills the next PSUM slot while ScalarE/VectorE are
        # busy -- removes the per-block pipeline bubble.
        jobs = [(jb, ci) for jb in range(NB) for ci in range(NCHUNK)]
        d_tiles = {}

        def emit_d(jb, ci):
            Dp = ps.tile([P, CHUNK], f32, tag="ps")
            for s0 in range(0, CHUNK, 512):
                nc.tensor.matmul(
                    Dp[:, s0:s0 + 512],
                    A_sb[:, jb * P:(jb + 1) * P],
                    B_sb[:, ci * CHUNK + s0: ci * CHUNK + s0 + 512],
                    start=True, stop=True)
            d_tiles[(jb, ci)] = Dp

        emit_d(*jobs[0])
        s_all = None
        for t, (jb, ci) in enumerate(jobs):
            if ci == 0:
                s_all = sb.tile([P, KRBF * NCHUNK], f32, tag="s_all")
                oh = sb.tile([P, P], f32, tag="oh")
                nc.vector.tensor_scalar(oh[:], iota_f[:], AC_sb[:, jb:jb + 1],
                                        None, Alu.is_equal)
            if t + 1 < len(jobs):
                emit_d(*jobs[t + 1])
            Dp = d_tiles.pop((jb, ci))
            cur = {}
            if chains:
                r_t = r_pool.tile([P, CHUNK], bf16, tag="r")
                nc.scalar.activation(r_t[:], Dp[:], Act.Exp, scale=-dg)
            for kind, k in plan:
                col = k * NCHUNK + ci
                if kind == "a":
                    t_rbf = rbfa_pool.tile([P, CHUNK], bf16, tag="rbf_a")
                    nc.scalar.activation(t_rbf[:], Dp[:], Act.Exp,
                                         scale=-g[k],
                                         accum_out=s_all[:, col:col + 1])
                else:
                    # NB: tensor_tensor_reduce + accum_out crashes the
                    # device in this pattern; scalar_tensor_tensor with
                    # accum_out computes the same product + row-sum and
                    # runs clean.  All-bf16 operands keep the DVE read
                    # ports free so it runs at full rate; the row-sum is
                    # accumulated from the fp32 ALU result.
                    t_rbf = rbfc_pool.tile([P, CHUNK], bf16, tag="rbf_c")
                    nc.vector.scalar_tensor_tensor(
                        out=t_rbf[:], in0=cur[k - 1][:], scalar=1.0,
                        in1=r_t[:], op0=Alu.mult, op1=Alu.mult,
                        accum_out=s_all[:, col:col + 1])
                cur[k] = t_rbf
            if ci == NCHUNK - 1:
                s17 = sb.tile([P, KRBF + 1], f32, tag="s17")
                nc.vector.tensor_reduce(
                    s17[:, 0:KRBF],
                    s_all[:].rearrange("p (k c) -> p k c", c=NCHUNK),
                    axis=mybir.AxisListType.X, op=Alu.add)
                nc.vector.memset(s17[:, KRBF:KRBF + 1], 1.0)
                hist = ps.tile([P, KRBF + 1], f32, tag="ps")
                nc.tensor.matmul(hist[:], oh[:], s17[:], start=True, stop=True)
                if jb == 0:
                    nc.vector.tensor_copy(G_sb[:], hist[:])
                else:
                    nc.vector.tensor_tensor(out=G_sb[:], in0=G_sb[:],
                                            in1=hist[:], op=Alu.add)

        # ---- epilogue: res = G^T @ T; build the stacked f-vector on 128
        # partitions (two transposes of res into one PSUM tile, then the
        # "- sumE" correction per lane); final linear = 9 K=128 matmuls
        # against the host-paired W128, all f32. ----
        res = ps.tile([KRBF + 1, EMB], f32, tag="ps")
        nc.tensor.matmul(res[:], G_sb[:], T_sb[:], start=True, stop=True)
        res_sb = sb.tile([KRBF + 1, EMB], f32, tag="res")
        nc.vector.tensor_copy(res_sb[:], res[:])
        rT = ps.tile([EMB, KRBF + 1], f32, tag="ps")
        nc.tensor.transpose(rT[:], res_sb[:], ident[0:KRBF + 1, 0:KRBF + 1])
        # duplicate res^T onto partitions 64..127 (transpose matmuls must
        # land on partition 0, so the upper copy goes via a tiny DMA)
        rT2 = sb.tile([P, KRBF + 1], f32, tag="rT2")
        nc.vector.tensor_copy(rT2[0:EMB, :], rT[:])
        rT_sb = sb.tile([EMB, KRBF + 1], f32, tag="rTsb")
        nc.vector.tensor_copy(rT_sb[:], rT[:])
        nc.sync.dma_start(rT2[EMB:P, :], rT_sb[:])
        f128 = sb.tile([P, 9], f32, tag="f128")
        even = rT2[0:EMB, 0:KRBF].rearrange("p (k two) -> p two k", two=2)
        odd = rT2[EMB:P, 0:KRBF].rearrange("p (k two) -> p two k", two=2)
        nc.vector.tensor_scalar(f128[0:EMB, 0:8], even[:, 0, :],
                                rT2[0:EMB, KRBF:KRBF + 1], None, Alu.subtract)
        nc.vector.tensor_copy(f128[0:EMB, 8:9], rT2[0:EMB, KRBF:KRBF + 1])
        nc.vector.tensor_scalar(f128[EMB:P, 0:8], odd[:, 1, :],
                                rT2[EMB:P, KRBF:KRBF + 1], None, Alu.subtract)
        nc.vector.memset(f128[EMB:P, 8:9], 0.0)

        outp = ps.tile([1, MD], f32, tag="ps")
        for cc in range(9):
            nc.tensor.matmul(outp[:],
                             f128[:, cc:cc + 1],
                             W_sb[:, cc * MD:(cc + 1) * MD],
                             start=(cc == 0), stop=(cc == 8))
        bsc = sb.tile([1, MD], f32, tag="bsc")
        nc.scalar.mul(bsc[:], b_sb[:], float(N) / NCORES)
        out_sb = sb.tile([1, MD], f32, tag="outsb")
        nc.vector.tensor_tensor(out=out_sb[:], in0=outp[:], in1=bsc[:],
                                op=Alu.add)
        nc.sync.dma_start(out_ext[:, :], out_sb[:])

    nc.compile()
    return nc


def _install_ntff_hook_shim():
    """Provide antenv.axon_hooks if the image's antenv lacks it.

    concourse's trace path (run_bass_kernel_spmd(trace=True) under axon)
    imports get_axon_ntff_profile_hook from there; the hook drives NRT
    profiling through libaxon_pjrt.so's C ABI (same contract the boot
    script uses)."""
    try:
        from antenv.axon_hooks import get_axon_ntff_profile_hook  # noqa: F401
        return
    except ImportError:
        pass
    import contextlib
    import ctypes
    import types

    so_path = os.environ.get("PJRT_LIBRARY_PATH", "/opt/axon/libaxon_pjrt.so")
    hook = None
    try:
        lib = ctypes.CDLL(so_path)
        if hasattr(lib, "axon_start_nrt_profile"):
            lib.axon_start_nrt_profile.argtypes = [
                ctypes.POINTER(ctypes.c_int64), ctypes.c_size_t]
            lib.axon_start_nrt_profile.restype = ctypes.c_int64
            lib.axon_stop_nrt_profile.argtypes = [ctypes.c_char_p]
            lib.axon_stop_nrt_profile.restype = ctypes.c_int64

            @contextlib.contextmanager
            def _hook(output_dir, device_ids):
                import jax
                jax.devices()
                if device_ids:
                    ids = (ctypes.c_int64 * len(device_ids))(*device_ids)
                    rc = lib.axon_start_nrt_profile(ids, len(device_ids))
                else:
                    rc = lib.axon_start_nrt_profile(None, 0)
                if rc != 0:
                    raise RuntimeError(f"axon_start_nrt_profile rc={rc}")
                try:
                    yield
                finally:
                    n = lib.axon_stop_nrt_profile(str(output_dir).encode())
                    print(f"ntff profile: {n} file(s) -> {output_dir}",
                          file=sys.stderr)

            hook = _hook
    except OSError:
        hook = None

    import antenv
    mod = types.ModuleType("antenv.axon_hooks")
    mod._hook = hook
    mod.get_axon_ntff_profile_hook = lambda: mod._hook

    def _set(h):
        mod._hook = h

    mod.set_axon_ntff_profile_hook = _set
    sys.modules["antenv.axon_hooks"] = mod
    antenv.axon_hooks = mod


def _run(inputs, trace=False):
    """Build + run on 8 NeuronCores. Returns (out[512] f32, exec_time_ns|None)."""
    _import_concourse()
    if trace:
        _install_ntff_hook_shim()
        from concourse import bass_utils as _bu
        _bu.upload_artifacts = lambda tmpdir: "local://" + str(tmpdir)
    from concourse.bass_utils import run_bass_kernel_spmd

    gamma = np.asarray(inputs["gamma"], np.float32).reshape(-1)
    anchors, chains, dg = _plan_k(gamma)
    in_maps = _prepare_in_maps(inputs["atomic_numbers"], inputs["positions"],
                               inputs["embed_table"], inputs["W"], inputs["b"])
    nc = _build(gamma, anchors, chains, dg)
    res = run_bass_kernel_spmd(nc, in_maps, core_ids=list(range(NCORES)),
                               trace=trace)
    out = np.zeros(MD, np.float32)
    for r in res.results:
        out += np.asarray(r["out"], np.float32).reshape(-1)
    return out, res.exec_time_ns


def kernel(**inputs) -> np.ndarray:
    out, _ = _run(inputs, trace=False)
    return out


# revision 14
# speedup vs baseline: 1.5087x; 1.1923x over previous
"""Trainium2 Bass kernel: GNN message passing (RBF all-pairs + embed einsum + linear).

Strategy (8 NeuronCores, SPMD):
  The reference output is atom_features.sum(axis=0) -- linear in everything
  after the RBF tensor -- so the whole network collapses to:
      s[j,k]  = sum_i exp(-gamma_k * ||pc_i - pc_j||^2)        (incl. i==j)
      ns[k,:] = sum_j (s[j,k] - 1) * E[j,:]                     (diag removed)
      out     = concat(sumE, ns.flatten()) @ W + N*b
  Each core owns a 512-row j-slice, computes s over all i via one augmented
  matmul (distance) + exp on ScalarE / chained multiplies on VectorE (fused
  row-reduce), histograms s against the one-hot of atomic numbers on the
  TensorEngine, and applies the embedding + final linear locally.  Per-core
  [512] partial outputs sum to the exact full result (host-side unshard).
"""

import os
import sys

import numpy as np

N = 4096
NCORES = 8
JC = N // NCORES          # j rows per core
P = 128
NB = JC // P              # j-blocks of 128 partitions per core
KRBF = 16
EMB = 64
VOCAB = 128
MD = 512
FIN = EMB + EMB * KRBF    # 1088
CHUNK = 2048              # i-chunk (PSUM: [128, 2048] f32 = 4 banks)
NCHUNK = N // CHUNK
ANCHOR_STRIDE = 2         # every ANCHOR_STRIDE-th k via ScalarE exp, rest chained on DVE


def _import_concourse():
    try:
        import concourse  # noqa: F401
        return
    except ImportError:
        pass
    for p in ("/opt/trn_rl_repo", "/root/.axon_site/_ro/trn_rl_repo"):
        if os.path.isdir(os.path.join(p, "concourse")):
            sys.path.insert(0, p)
            import concourse  # noqa: F401
            return
    raise ImportError("cannot locate the concourse (bass) package")


def _plan_k(gamma: np.ndarray):
    """Anchor/chain split of the 16 RBF exponents.

    If gamma is uniformly spaced (it is: linspace), odd k's are computed on
    the VectorEngine as rbf[k] = rbf[k-1] * exp(-dg*D); otherwise fall back
    to 16 direct exps on ScalarE."""
    g = np.asarray(gamma, np.float64).reshape(-1)
    assert g.shape[0] == KRBF
    dif = np.diff(g)
    if np.allclose(dif, dif[0], rtol=1e-5, atol=1e-7):
        anchors = list(range(0, KRBF, ANCHOR_STRIDE))
        chains = [k for k in range(KRBF) if k not in anchors]
        return anchors, chains, float(dif[0])
    return list(range(KRBF)), [], 0.0


def _prepare_in_maps(atomic_numbers, positions, embed_table, W, b):
    pos = np.asarray(positions, np.float32)
    pc = pos - pos.mean(axis=0, keepdims=True)
    nrm = (pc * pc).sum(axis=1).astype(np.float32)
    ones = np.ones(N, np.float32)
    A_full = np.ascontiguousarray(
        np.stack([pc[:, 0], pc[:, 1], pc[:, 2], nrm, ones]))            # [5, N]
    B_full = np.ascontiguousarray(
        np.stack([-2 * pc[:, 0], -2 * pc[:, 1], -2 * pc[:, 2], ones, nrm]))
    a = np.asarray(atomic_numbers).astype(np.int64).reshape(N)
    T = np.ascontiguousarray(np.asarray(embed_table, np.float32))
    Wf = np.asarray(W, np.float32)
    # W128: pair the 17 64-row f-chunks (ns_0..ns_15, sumE) onto 128
    # partitions so the final linear is 9 K=128 matmuls instead of 17 K=64.
    # f-chunk i: ns_i -> W rows [64+64i, 128+64i); sumE -> W rows [0, 64).
    def _wchunk(i):
        if i < KRBF:
            return Wf[EMB + EMB * i: 2 * EMB + EMB * i, :]
        if i == KRBF:
            return Wf[0:EMB, :]
        return np.zeros((EMB, MD), np.float32)
    W128 = np.empty((P, 9 * MD), np.float32)
    for c in range(9):
        W128[0:EMB, c * MD:(c + 1) * MD] = _wchunk(2 * c)
        W128[EMB:P, c * MD:(c + 1) * MD] = _wchunk(2 * c + 1)
    W128 = np.ascontiguousarray(W128)
    bf = np.ascontiguousarray(np.asarray(b, np.float32).reshape(1, MD))
    in_maps = []
    for c in range(NCORES):
        js = slice(c * JC, (c + 1) * JC)
        ac = np.ascontiguousarray(
            a[js].reshape(NB, P).T.astype(np.float32))                  # [128, NB]
        in_maps.append({
            "A": np.ascontiguousarray(A_full[:, js]),
            "B": B_full,
            "AC": ac,
            "T": T,
            "W": W128,
            "BB": bf,
        })
    return in_maps


def _build(gamma, anchors, chains, dg):
    from contextlib import ExitStack

    import concourse.mybir as mybir
    import concourse.tile as tile
    from concourse import bacc
    from concourse.masks import make_identity

    f32 = mybir.dt.float32
    bf16 = mybir.dt.bfloat16
    Alu = mybir.AluOpType
    Act = mybir.ActivationFunctionType

    g = [float(x) for x in np.asarray(gamma, np.float64).reshape(-1)]

    nc = bacc.Bacc("TRN2", target_bir_lowering=False, debug=False,
                   num_devices=NCORES)

    A_ext = nc.declare_dram_parameter("A", [5, JC], f32, isOutput=False)
    B_ext = nc.declare_dram_parameter("B", [5, N], f32, isOutput=False)
    AC_ext = nc.declare_dram_parameter("AC", [P, NB], f32, isOutput=False)
    T_ext = nc.declare_dram_parameter("T", [VOCAB, EMB], f32, isOutput=False)
    W_ext = nc.declare_dram_parameter("W", [P, 9 * MD], f32, isOutput=False)
    BB_ext = nc.declare_dram_parameter("BB", [1, MD], f32, isOutput=False)
    out_ext = nc.declare_dram_parameter("out", [1, MD], f32, isOutput=True)

    with tile.TileContext(nc) as tc, ExitStack() as ctx:
        consts = ctx.enter_context(tc.tile_pool(name="consts", bufs=1))
        sb = ctx.enter_context(tc.tile_pool(name="sb", bufs=2))
        # NOTE: 8 accum-activations per chunk rotating through fewer SBUF
        # slots than anchors-in-flight crashed the exec unit on hardware
        # (NRT_EXEC_UNIT_UNRECOVERABLE); bufs >= anchors-per-chunk avoids
        # intra-chunk slot reuse and runs clean.  Chain outputs are pure
        # sinks (the row-sum rides the fp32 ALU result), so they can be
        # bf16 to save SBUF.
        rbfa_pool = ctx.enter_context(
            tc.tile_pool(name="rbfa", bufs=max(8, len(anchors))))
        rbfc_pool = ctx.enter_context(tc.tile_pool(name="rbfc", bufs=10))
        r_pool = ctx.enter_context(tc.tile_pool(name="rp", bufs=2))
        ps = ctx.enter_context(tc.tile_pool(name="ps", bufs=2, space="PSUM"))

        # ---- inputs / constants ----
        A_sb = consts.tile([5, JC], f32)
        nc.sync.dma_start(A_sb[:], A_ext[:, :])
        B_sb = consts.tile([5, N], f32)
        nc.sync.dma_start(B_sb[:], B_ext[:, :])
        AC_sb = consts.tile([P, NB], f32)
        nc.sync.dma_start(AC_sb[:], AC_ext[:, :])
        T_sb = consts.tile([VOCAB, EMB], f32)
        nc.sync.dma_start(T_sb[:], T_ext[:, :])
        W_sb = consts.tile([P, 9 * MD], f32)
        nc.sync.dma_start(W_sb[:], W_ext[:, :])
        b_sb = consts.tile([1, MD], f32)
        nc.sync.dma_start(b_sb[:], BB_ext[:, :])

        iota_i = consts.tile([P, P], mybir.dt.int32)
        nc.gpsimd.iota(iota_i[:], pattern=[[1, P]], base=0, channel_multiplier=0)
        iota_f = consts.tile([P, P], f32)
        nc.vector.tensor_copy(iota_f[:], iota_i[:])
        ident = consts.tile([32, 32], f32)
        make_identity(nc, ident[:])
        G_sb = consts.tile([P, KRBF + 1], f32)
        # Per-k schedule: each anchor is followed by the chain run it feeds
        # (chain k multiplies k-1's tile by r on the VectorEngine).
        plan = []
        for k in range(KRBF):
            plan.append(("a" if k in anchors else "c", k))

        # ---- main loop over (j-block, i-chunk) jobs ----
        # D for job t+1 is emitted before job t's exp/chain work so the
        # TensorEngine fills the next PSUM slot while ScalarE/VectorE are
        # busy -- removes the per-block pipeline bubble.
        jobs = [(jb, ci) for jb in range(NB) for ci in range(NCHUNK)]
        d_tiles = {}

        def emit_d(jb, ci):
            Dp = ps.tile([P, CHUNK], f32, tag="ps")
            for s0 in range(0, CHUNK, 512):
                nc.tensor.matmul(
                    Dp[:, s0:s0 + 512],
                    A_sb[:, jb * P:(jb + 1) * P],
                    B_sb[:, ci * CHUNK + s0: ci * CHUNK + s0 + 512],
                    start=True, stop=True)
            d_tiles[(jb, ci)] = Dp

        emit_d(*jobs[0])
        s_all = None
        for t, (jb, ci) in enumerate(jobs):
            if ci == 0:
                s_all = sb.tile([P, KRBF * NCHUNK], f32, tag="s_all")
                oh = sb.tile([P, P], f32, tag="oh")
                nc.vector.tensor_scalar(oh[:], iota_f[:], AC_sb[:, jb:jb + 1],
                                        None, Alu.is_equal)
            if t + 1 < len(jobs):
                emit_d(*jobs[t + 1])
            Dp = d_tiles.pop((jb, ci))
            cur = {}
            if chains:
                r_t = r_pool.tile([P, CHUNK], bf16, tag="r")
                nc.scalar.activation(r_t[:], Dp[:], Act.Exp, scale=-dg)
            for kind, k in plan:
                col = k * NCHUNK + ci
                if kind == "a":
                    t_rbf = rbfa_pool.tile([P, CHUNK], bf16, tag="rbf_a")
                    nc.scalar.activation(t_rbf[:], Dp[:], Act.Exp,
                                         scale=-g[k],
                                         accum_out=s_all[:, col:col + 1])
                else:
                    # NB: tensor_tensor_reduce + accum_out crashes the
                    # device in this pattern; scalar_tensor_tensor with
                    # accum_out computes the same product + row-sum and
                    # runs clean.  All-bf16 operands keep the DVE read
                    # ports free so it runs at full rate; the row-sum is
                    # accumulated from the fp32 ALU result.
                    t_rbf = rbfc_pool.tile([P, CHUNK], bf16, tag="rbf_c")
                    nc.vector.scalar_tensor_tensor(
                        out=t_rbf[:], in0=cur[k - 1][:], scalar=1.0,
                        in1=r_t[:], op0=Alu.mult, op1=Alu.mult,
                        accum_out=s_all[:, col:col + 1])
                cur[k] = t_rbf
            if ci == NCHUNK - 1:
                s17 = sb.tile([P, KRBF + 1], f32, tag="s17")
                nc.vector.tensor_reduce(
                    s17[:, 0:KRBF],
                    s_all[:].rearrange("p (k c) -> p k c", c=NCHUNK),
                    axis=mybir.AxisListType.X, op=Alu.add)
                nc.vector.memset(s17[:, KRBF:KRBF + 1], 1.0)
                hist = ps.tile([P, KRBF + 1], f32, tag="ps")
                nc.tensor.matmul(hist[:], oh[:], s17[:], start=True, stop=True)
                if jb == 0:
                    nc.vector.tensor_copy(G_sb[:], hist[:])
                else:
                    nc.vector.tensor_tensor(out=G_sb[:], in0=G_sb[:],
                                            in1=hist[:], op=Alu.add)

        # ---- epilogue: res = G^T @ T; build the stacked f-vector on 128
        # partitions (two transposes of res into one PSUM tile, then the
        # "- sumE" correction per lane); final linear = 9 K=128 matmuls
        # against the host-paired W128, all f32. ----
        res = ps.tile([KRBF + 1, EMB], f32, tag="ps")
        nc.tensor.matmul(res[:], G_sb[:], T_sb[:], start=True, stop=True)
        res_sb = sb.tile([KRBF + 1, EMB], f32, tag="res")
        nc.vector.tensor_copy(res_sb[:], res[:])
        rT = ps.tile([EMB, KRBF + 1], f32, tag="ps")
        nc.tensor.transpose(rT[:], res_sb[:], ident[0:KRBF + 1, 0:KRBF + 1])
        # duplicate res^T onto partitions 64..127 (transpose matmuls must
        # land on partition 0, so the upper copy goes via a tiny DMA)
        rT2 = sb.tile([P, KRBF + 1], f32, tag="rT2")
        nc.vector.tensor_copy(rT2[0:EMB, :], rT[:])
        rT_sb = sb.tile([EMB, KRBF + 1], f32, tag="rTsb")
        nc.vector.tensor_copy(rT_sb[:], rT[:])
        nc.sync.dma_start(rT2[EMB:P, :], rT_sb[:])
        f128 = sb.tile([P, 9], f32, tag="f128")
        even = rT2[0:EMB, 0:KRBF].rearrange("p (k two) -> p two k", two=2)
        odd = rT2[EMB:P, 0:KRBF].rearrange("p (k two) -> p two k", two=2)
        nc.vector.tensor_scalar(f128[0:EMB, 0:8], even[:, 0, :],
                                rT2[0:EMB, KRBF:KRBF + 1], None, Alu.subtract)
        nc.vector.tensor_copy(f128[0:EMB, 8:9], rT2[0:EMB, KRBF:KRBF + 1])
        nc.vector.tensor_scalar(f128[EMB:P, 0:8], odd[:, 1, :],
                                rT2[EMB:P, KRBF:KRBF + 1], None, Alu.subtract)
        nc.vector.memset(f128[EMB:P, 8:9], 0.0)

        outp = ps.tile([1, MD], f32, tag="ps")
        for cc in range(9):
            nc.tensor.matmul(outp[:],
                             f128[:, cc:cc + 1],
                             W_sb[:, cc * MD:(cc + 1) * MD],
                             start=(cc == 0), stop=(cc == 8))
        bsc = sb.tile([1, MD], f32, tag="bsc")
        nc.scalar.mul(bsc[:], b_sb[:], float(N) / NCORES)
        out_sb = sb.tile([1, MD], f32, tag="outsb")
        nc.vector.tensor_tensor(out=out_sb[:], in0=outp[:], in1=bsc[:],
                                op=Alu.add)
        nc.sync.dma_start(out_ext[:, :], out_sb[:])

    nc.compile()
    return nc


def _install_ntff_hook_shim():
    """Provide antenv.axon_hooks if the image's antenv lacks it.

    concourse's trace path (run_bass_kernel_spmd(trace=True) under axon)
    imports get_axon_ntff_profile_hook from there; the hook drives NRT
    profiling through libaxon_pjrt.so's C ABI (same contract the boot
    script uses)."""
    try:
        from antenv.axon_hooks import get_axon_ntff_profile_hook  # noqa: F401
        return
    except ImportError:
        pass
    import contextlib
    import ctypes
    import types

    so_path = os.environ.get("PJRT_LIBRARY_PATH", "/opt/axon/libaxon_pjrt.so")
    hook = None
    try:
        lib = ctypes.CDLL(so_path)
        if hasattr(lib, "axon_start_nrt_profile"):
            lib.axon_start_nrt_profile.argtypes = [
                ctypes.POINTER(ctypes.c_int64), ctypes.c_size_t]
            lib.axon_start_nrt_profile.restype = ctypes.c_int64
            lib.axon_stop_nrt_profile.argtypes = [ctypes.c_char_p]
            lib.axon_stop_nrt_profile.restype = ctypes.c_int64

            @contextlib.contextmanager
            def _hook(output_dir, device_ids):
                import jax
                jax.devices()
                if device_ids:
                    ids = (ctypes.c_int64 * len(device_ids))(*device_ids)
                    rc = lib.axon_start_nrt_profile(ids, len(device_ids))
                else:
                    rc = lib.axon_start_nrt_profile(None, 0)
                if rc != 0:
                    raise RuntimeError(f"axon_start_nrt_profile rc={rc}")
                try:
                    yield
                finally:
                    n = lib.axon_stop_nrt_profile(str(output_dir).encode())
                    print(f"ntff profile: {n} file(s) -> {output_dir}",
                          file=sys.stderr)

            hook = _hook
    except OSError:
        hook = None

    import antenv
    mod = types.ModuleType("antenv.axon_hooks")
    mod._hook = hook
    mod.get_axon_ntff_profile_hook = lambda: mod._hook

    def _set(h):
        mod._hook = h

    mod.set_axon_ntff_profile_hook = _set
    sys.modules["antenv.axon_hooks"] = mod
    antenv.axon_hooks = mod


def _run(inputs, trace=False):
    """Build + run on 8 NeuronCores. Returns (out[512] f32, exec_time_ns|None)."""
    _import_concourse()
    if trace:
        _install_ntff_hook_shim()
        from concourse import bass_utils as _bu
        _bu.upload_artifacts = lambda tmpdir: "local://" + str(tmpdir)
    from concourse.bass_utils import run_bass_kernel_spmd

    gamma = np.asarray(inputs["gamma"], np.float32).reshape(-1)
    anchors, chains, dg = _plan_k(gamma)
    in_maps = _prepare_in_maps(inputs["atomic_numbers"], inputs["positions"],
                               inputs["embed_table"], inputs["W"], inputs["b"])
    nc = _build(gamma, anchors, chains, dg)
    res = run_bass_kernel_spmd(nc, in_maps, core_ids=list(range(NCORES)),
                               trace=trace)
    out = np.zeros(MD, np.float32)
    for r in res.results:
        out += np.asarray(r["out"], np.float32).reshape(-1)
    return out, res.exec_time_ns


def kernel(**inputs) -> np.ndarray:
    out, _ = _run(inputs, trace=False)
    return out


# revision 15
# speedup vs baseline: 1.5092x; 1.0003x over previous
"""Trainium2 Bass kernel: GNN message passing (RBF all-pairs + embed einsum + linear).

Strategy (8 NeuronCores, SPMD):
  The reference output is atom_features.sum(axis=0) -- linear in everything
  after the RBF tensor -- so the whole network collapses to:
      s[j,k]  = sum_i exp(-gamma_k * ||pc_i - pc_j||^2)        (incl. i==j)
      ns[k,:] = sum_j (s[j,k] - 1) * E[j,:]                     (diag removed)
      out     = concat(sumE, ns.flatten()) @ W + N*b
  Each core owns a 512-row j-slice, computes s over all i via one augmented
  matmul (distance) + exp on ScalarE / chained multiplies on VectorE (fused
  row-reduce), histograms s against the one-hot of atomic numbers on the
  TensorEngine, and applies the embedding + final linear locally.  Per-core
  [512] partial outputs sum to the exact full result (host-side unshard).
"""

import os
import sys

import numpy as np

N = 4096
NCORES = 8
JC = N // NCORES          # j rows per core
P = 128
NB = JC // P              # j-blocks of 128 partitions per core
KRBF = 16
EMB = 64
VOCAB = 128
MD = 512
FIN = EMB + EMB * KRBF    # 1088
CHUNK = 2048              # i-chunk (PSUM: [128, 2048] f32 = 4 banks)
NCHUNK = N // CHUNK
ANCHOR_STRIDE = 2         # every ANCHOR_STRIDE-th k via ScalarE exp, rest chained on DVE


def _import_concourse():
    try:
        import concourse  # noqa: F401
        return
    except ImportError:
        pass
    for p in ("/opt/trn_rl_repo", "/root/.axon_site/_ro/trn_rl_repo"):
        if os.path.isdir(os.path.join(p, "concourse")):
            sys.path.insert(0, p)
            import concourse  # noqa: F401
            return
    raise ImportError("cannot locate the concourse (bass) package")


def _plan_k(gamma: np.ndarray):
    """Anchor/chain split of the 16 RBF exponents.

    If gamma is uniformly spaced (it is: linspace), odd k's are computed on
    the VectorEngine as rbf[k] = rbf[k-1] * exp(-dg*D); otherwise fall back
    to 16 direct exps on ScalarE."""
    g = np.asarray(gamma, np.float64).reshape(-1)
    assert g.shape[0] == KRBF
    dif = np.diff(g)
    if np.allclose(dif, dif[0], rtol=1e-5, atol=1e-7):
        anchors = list(range(0, KRBF, ANCHOR_STRIDE))
        chains = [k for k in range(KRBF) if k not in anchors]
        return anchors, chains, float(dif[0])
    return list(range(KRBF)), [], 0.0


def _prepare_in_maps(atomic_numbers, positions, embed_table, W, b):
    pos = np.asarray(positions, np.float32)
    pc = pos - pos.mean(axis=0, keepdims=True)
    nrm = (pc * pc).sum(axis=1).astype(np.float32)
    ones = np.ones(N, np.float32)
    A_full = np.ascontiguousarray(
        np.stack([pc[:, 0], pc[:, 1], pc[:, 2], nrm, ones]))            # [5, N]
    B_full = np.ascontiguousarray(
        np.stack([-2 * pc[:, 0], -2 * pc[:, 1], -2 * pc[:, 2], ones, nrm]))
    a = np.asarray(atomic_numbers).astype(np.int64).reshape(N)
    T = np.ascontiguousarray(np.asarray(embed_table, np.float32))
    Wf = np.asarray(W, np.float32)
    # W128: pair the 17 64-row f-chunks (ns_0..ns_15, sumE) onto 128
    # partitions so the final linear is 9 K=128 matmuls instead of 17 K=64.
    # f-chunk i: ns_i -> W rows [64+64i, 128+64i); sumE -> W rows [0, 64).
    def _wchunk(i):
        if i < KRBF:
            return Wf[EMB + EMB * i: 2 * EMB + EMB * i, :]
        if i == KRBF:
            return Wf[0:EMB, :]
        return np.zeros((EMB, MD), np.float32)
    W128 = np.empty((P, 9 * MD), np.float32)
    for c in range(9):
        W128[0:EMB, c * MD:(c + 1) * MD] = _wchunk(2 * c)
        W128[EMB:P, c * MD:(c + 1) * MD] = _wchunk(2 * c + 1)
    W128 = np.ascontiguousarray(W128)
    bf = np.ascontiguousarray(np.asarray(b, np.float32).reshape(1, MD))
    in_maps = []
    for c in range(NCORES):
        js = slice(c * JC, (c + 1) * JC)
        ac = np.ascontiguousarray(
            a[js].reshape(NB, P).T.astype(np.float32))                  # [128, NB]
        in_maps.append({
            "A": np.ascontiguousarray(A_full[:, js]),
            "B": B_full,
            "AC": ac,
            "T": T,
            "W": W128,
            "BB": bf,
        })
    return in_maps


def _build(gamma, anchors, chains, dg):
    from contextlib import ExitStack

    import concourse.mybir as mybir
    import concourse.tile as tile
    from concourse import bacc
    from concourse.masks import make_identity

    f32 = mybir.dt.float32
    bf16 = mybir.dt.bfloat16
    Alu = mybir.AluOpType
    Act = mybir.ActivationFunctionType

    g = [float(x) for x in np.asarray(gamma, np.float64).reshape(-1)]

    nc = bacc.Bacc("TRN2", target_bir_lowering=False, debug=False,
                   num_devices=NCORES)

    A_ext = nc.declare_dram_parameter("A", [5, JC], f32, isOutput=False)
    B_ext = nc.declare_dram_parameter("B", [5, N], f32, isOutput=False)
    AC_ext = nc.declare_dram_parameter("AC", [P, NB], f32, isOutput=False)
    T_ext = nc.declare_dram_parameter("T", [VOCAB, EMB], f32, isOutput=False)
    W_ext = nc.declare_dram_parameter("W", [P, 9 * MD], f32, isOutput=False)
    BB_ext = nc.declare_dram_parameter("BB", [1, MD], f32, isOutput=False)
    out_ext = nc.declare_dram_parameter("out", [1, MD], f32, isOutput=True)

    with tile.TileContext(nc) as tc, ExitStack() as ctx:
        consts = ctx.enter_context(tc.tile_pool(name="consts", bufs=1))
        sb = ctx.enter_context(tc.tile_pool(name="sb", bufs=2))
        # NOTE: 8 accum-activations per chunk rotating through fewer SBUF
        # slots than anchors-in-flight crashed the exec unit on hardware
        # (NRT_EXEC_UNIT_UNRECOVERABLE); bufs >= anchors-per-chunk avoids
        # intra-chunk slot reuse and runs clean.  Chain outputs are pure
        # sinks (the row-sum rides the fp32 ALU result), so they can be
        # bf16 to save SBUF.
        rbfa_pool = ctx.enter_context(
            tc.tile_pool(name="rbfa", bufs=max(8, len(anchors))))
        rbfc_pool = ctx.enter_context(tc.tile_pool(name="rbfc", bufs=10))
        r_pool = ctx.enter_context(tc.tile_pool(name="rp", bufs=2))
        ps = ctx.enter_context(tc.tile_pool(name="ps", bufs=2, space="PSUM"))

        # ---- inputs / constants ----
        A_sb = consts.tile([5, JC], f32)
        nc.sync.dma_start(A_sb[:], A_ext[:, :])
        B_sb = consts.tile([5, N], f32)
        nc.sync.dma_start(B_sb[:, 0:CHUNK], B_ext[:, 0:CHUNK])
        nc.sync.dma_start(B_sb[:, CHUNK:N], B_ext[:, CHUNK:N])
        AC_sb = consts.tile([P, NB], f32)
        nc.sync.dma_start(AC_sb[:], AC_ext[:, :])
        T_sb = consts.tile([VOCAB, EMB], f32)
        nc.sync.dma_start(T_sb[:], T_ext[:, :])
        W_sb = consts.tile([P, 9 * MD], f32)
        nc.sync.dma_start(W_sb[:], W_ext[:, :])
        b_sb = consts.tile([1, MD], f32)
        nc.sync.dma_start(b_sb[:], BB_ext[:, :])
        bsc = consts.tile([1, MD], f32)
        nc.scalar.mul(bsc[:], b_sb[:], float(N) / NCORES)

        iota_i = consts.tile([P, P], mybir.dt.int32)
        nc.gpsimd.iota(iota_i[:], pattern=[[1, P]], base=0, channel_multiplier=0)
        iota_f = consts.tile([P, P], f32)
        nc.vector.tensor_copy(iota_f[:], iota_i[:])
        ident = consts.tile([32, 32], f32)
        make_identity(nc, ident[:])
        G_sb = consts.tile([P, KRBF + 1], f32)
        # Per-k schedule: each anchor is followed by the chain run it feeds
        # (chain k multiplies k-1's tile by r on the VectorEngine).
        plan = []
        for k in range(KRBF):
            plan.append(("a" if k in anchors else "c", k))

        # ---- main loop over (j-block, i-chunk) jobs ----
        # D for job t+1 is emitted before job t's exp/chain work so the
        # TensorEngine fills the next PSUM slot while ScalarE/VectorE are
        # busy -- removes the per-block pipeline bubble.
        jobs = [(jb, ci) for jb in range(NB) for ci in range(NCHUNK)]
        d_tiles = {}

        def emit_d(jb, ci):
            Dp = ps.tile([P, CHUNK], f32, tag="ps")
            for s0 in range(0, CHUNK, 512):
                nc.tensor.matmul(
                    Dp[:, s0:s0 + 512],
                    A_sb[:, jb * P:(jb + 1) * P],
                    B_sb[:, ci * CHUNK + s0: ci * CHUNK + s0 + 512],
                    start=True, stop=True)
            d_tiles[(jb, ci)] = Dp

        emit_d(*jobs[0])
        s_all = None
        for t, (jb, ci) in enumerate(jobs):
            if ci == 0:
                s_all = sb.tile([P, KRBF * NCHUNK], f32, tag="s_all")
                oh = sb.tile([P, P], f32, tag="oh")
                nc.vector.tensor_scalar(oh[:], iota_f[:], AC_sb[:, jb:jb + 1],
                                        None, Alu.is_equal)
            if t + 1 < len(jobs):
                emit_d(*jobs[t + 1])
            Dp = d_tiles.pop((jb, ci))
            cur = {}
            if chains:
                r_t = r_pool.tile([P, CHUNK], bf16, tag="r")
                nc.scalar.activation(r_t[:], Dp[:], Act.Exp, scale=-dg)
            for kind, k in plan:
                col = k * NCHUNK + ci
                if kind == "a":
                    t_rbf = rbfa_pool.tile([P, CHUNK], bf16, tag="rbf_a")
                    nc.scalar.activation(t_rbf[:], Dp[:], Act.Exp,
                                         scale=-g[k],
                                         accum_out=s_all[:, col:col + 1])
                else:
                    # NB: tensor_tensor_reduce + accum_out crashes the
                    # device in this pattern; scalar_tensor_tensor with
                    # accum_out computes the same product + row-sum and
                    # runs clean.  All-bf16 operands keep the DVE read
                    # ports free so it runs at full rate; the row-sum is
                    # accumulated from the fp32 ALU result.
                    t_rbf = rbfc_pool.tile([P, CHUNK], bf16, tag="rbf_c")
                    nc.vector.scalar_tensor_tensor(
                        out=t_rbf[:], in0=cur[k - 1][:], scalar=1.0,
                        in1=r_t[:], op0=Alu.mult, op1=Alu.mult,
                        accum_out=s_all[:, col:col + 1])
                cur[k] = t_rbf
            if ci == NCHUNK - 1:
                s17 = sb.tile([P, KRBF + 1], f32, tag="s17")
                nc.vector.tensor_reduce(
                    s17[:, 0:KRBF],
                    s_all[:].rearrange("p (k c) -> p k c", c=NCHUNK),
                    axis=mybir.AxisListType.X, op=Alu.add)
                nc.vector.memset(s17[:, KRBF:KRBF + 1], 1.0)
                hist = ps.tile([P, KRBF + 1], f32, tag="ps")
                nc.tensor.matmul(hist[:], oh[:], s17[:], start=True, stop=True)
                if jb == 0:
                    nc.vector.tensor_copy(G_sb[:], hist[:])
                else:
                    nc.vector.tensor_tensor(out=G_sb[:], in0=G_sb[:],
                                            in1=hist[:], op=Alu.add)

        # ---- epilogue: res = G^T @ T; build the stacked f-vector on 128
        # partitions (two transposes of res into one PSUM tile, then the
        # "- sumE" correction per lane); final linear = 9 K=128 matmuls
        # against the host-paired W128, all f32. ----
        res = ps.tile([KRBF + 1, EMB], f32, tag="ps")
        nc.tensor.matmul(res[:], G_sb[:], T_sb[:], start=True, stop=True)
        res_sb = sb.tile([KRBF + 1, EMB], f32, tag="res")
        nc.vector.tensor_copy(res_sb[:], res[:])
        rT = ps.tile([EMB, KRBF + 1], f32, tag="ps")
        nc.tensor.transpose(rT[:], res_sb[:], ident[0:KRBF + 1, 0:KRBF + 1])
        # duplicate res^T onto partitions 64..127 (transpose matmuls must
        # land on partition 0, so the upper copy goes via a tiny DMA)
        rT2 = sb.tile([P, KRBF + 1], f32, tag="rT2")
        nc.vector.tensor_copy(rT2[0:EMB, :], rT[:])
        rT_sb = sb.tile([EMB, KRBF + 1], f32, tag="rTsb")
        nc.vector.tensor_copy(rT_sb[:], rT[:])
        nc.sync.dma_start(rT2[EMB:P, :], rT_sb[:])
        f128 = sb.tile([P, 9], f32, tag="f128")
        even = rT2[0:EMB, 0:KRBF].rearrange("p (k two) -> p two k", two=2)
        odd = rT2[EMB:P, 0:KRBF].rearrange("p (k two) -> p two k", two=2)
        nc.vector.tensor_scalar(f128[0:EMB, 0:8], even[:, 0, :],
                                rT2[0:EMB, KRBF:KRBF + 1], None, Alu.subtract)
        nc.vector.tensor_copy(f128[0:EMB, 8:9], rT2[0:EMB, KRBF:KRBF + 1])
        nc.vector.tensor_scalar(f128[EMB:P, 0:8], odd[:, 1, :],
                                rT2[EMB:P, KRBF:KRBF + 1], None, Alu.subtract)
        nc.vector.memset(f128[EMB:P, 8:9], 0.0)

        outp = ps.tile([1, MD], f32, tag="ps")
        for cc in range(9):
            nc.tensor.matmul(outp[:],
                             f128[:, cc:cc + 1],
                             W_sb[:, cc * MD:(cc + 1) * MD],
                             start=(cc == 0), stop=(cc == 8))
        out_sb = sb.tile([1, MD], f32, tag="outsb")
        nc.vector.tensor_tensor(out=out_sb[:], in0=outp[:], in1=bsc[:],
                                op=Alu.add)
        nc.sync.dma_start(out_ext[:, :], out_sb[:])

    nc.compile()
    return nc


def _install_ntff_hook_shim():
    """Provide antenv.axon_hooks if the image's antenv lacks it.

    concourse's trace path (run_bass_kernel_spmd(trace=True) under axon)
    imports get_axon_ntff_profile_hook from there; the hook drives NRT
    profiling through libaxon_pjrt.so's C ABI (same contract the boot
    script uses)."""
    try:
        from antenv.axon_hooks import get_axon_ntff_profile_hook  # noqa: F401
        return
    except ImportError:
        pass
    import contextlib
    import ctypes
    import types

    so_path = os.environ.get("PJRT_LIBRARY_PATH", "/opt/axon/libaxon_pjrt.so")
    hook = None
    try:
        lib = ctypes.CDLL(so_path)
        if hasattr(lib, "axon_start_nrt_profile"):
            lib.axon_start_nrt_profile.argtypes = [
                ctypes.POINTER(ctypes.c_int64), ctypes.c_size_t]
            lib.axon_start_nrt_profile.restype = ctypes.c_int64
            lib.axon_stop_nrt_profile.argtypes = [ctypes.c_char_p]
            lib.axon_stop_nrt_profile.restype = ctypes.c_int64

            @contextlib.contextmanager
            def _hook(output_dir, device_ids):
                import jax
                jax.devices()
                if device_ids:
                    ids = (ctypes.c_int64 * len(device_ids))(*device_ids)
                    rc = lib.axon_start_nrt_profile(ids, len(device_ids))
                else:
                    rc = lib.axon_start_nrt_profile(None, 0)
                if rc != 0:
                    raise RuntimeError(f"axon_start_nrt_profile rc={rc}")
                try:
                    yield
                finally:
                    n = lib.axon_stop_nrt_profile(str(output_dir).encode())
                    print(f"ntff profile: {n} file(s) -> {output_dir}",
                          file=sys.stderr)

            hook = _hook
    except OSError:
        hook = None

    import antenv
    mod = types.ModuleType("antenv.axon_hooks")
    mod._hook = hook
    mod.get_axon_ntff_profile_hook = lambda: mod._hook

    def _set(h):
        mod._hook = h

    mod.set_axon_ntff_profile_hook = _set
    sys.modules["antenv.axon_hooks"] = mod
    antenv.axon_hooks = mod


def _run(inputs, trace=False):
    """Build + run on 8 NeuronCores. Returns (out[512] f32, exec_time_ns|None)."""
    _import_concourse()
    if trace:
        _install_ntff_hook_shim()
        from concourse import bass_utils as _bu
        _bu.upload_artifacts = lambda tmpdir: "local://" + str(tmpdir)
    from concourse.bass_utils import run_bass_kernel_spmd

    gamma = np.asarray(inputs["gamma"], np.float32).reshape(-1)
    anchors, chains, dg = _plan_k(gamma)
    in_maps = _prepare_in_maps(inputs["atomic_numbers"], inputs["positions"],
                               inputs["embed_table"], inputs["W"], inputs["b"])
    nc = _build(gamma, anchors, chains, dg)
    res = run_bass_kernel_spmd(nc, in_maps, core_ids=list(range(NCORES)),
                               trace=trace)
    out = np.zeros(MD, np.float32)
    for r in res.results:
        out += np.asarray(r["out"], np.float32).reshape(-1)
    return out, res.exec_time_ns


def kernel(**inputs) -> np.ndarray:
    out, _ = _run(inputs, trace=False)
    return out
